# revision 32
# baseline (speedup 1.0000x reference)
"""ResNet bottleneck block (1x1 -> 3x3 -> 1x1 convs, folded BN, residual ReLU)
on 8 Trainium2 NeuronCores, data-parallel over the batch dim.

Layout strategy (per core, 8 images):
  - x arranged [img, p, kc, hw] so each image DMA is one contiguous
    [128, 8*784] transfer; channel c = kc*128 + p.
  - All matmul operands are bf16 (weights folded+cast host-side, x cast
    host-side): bf16 LDWEIGHTS uses the fast-weight-load path (~53ns) and
    hides behind the 163ns matmul stream, where f32r's 2-pass ~225ns weight
    load serialized ahead of every matmul and set the PE cadence.
  - 1x1 convs are matmuls over the flattened spatial dim (N split 2x392).
  - 3x3 conv = 9 shifted matmuls accumulating in PSUM, reading a
    zero-padded 30x30 SBUF image so every matmul is a uniform [128,14,28].
  - BN scale folded into the weights host-side; BN shift + ReLU fused into
    the PSUM->SBUF evacuation on ScalarE. conv3's residual-add runs as a
    scalar_tensor_tensor on VectorE, final ReLU on ScalarE.
  - Output stored bf16 and upcast to f32 host-side (halves the store DMA).
  - Software pipeline over images: DMA(t) / conv1(t-1) / conv2(t-2) /
    conv3+store(t-3) so the PE stream never waits on a same-image epilogue.
"""

import math
import os

import numpy as np
import ml_dtypes

import concourse.bass as bass
import concourse.mybir as mybir
import concourse.tile as tile
from concourse.bass_utils import run_bass_kernel_spmd

# Problem constants (hardcoded per the grading contract).
B, CIN, H, W = 64, 1024, 28, 28
WIDTH, COUT = 256, 1024
NCORES = 8
BPC = B // NCORES          # images per core
S = H * W                  # 784
PW = W + 2                 # 30 (padded row width)
PS = PW * PW               # 900
NROW = H // 2              # 14 rows per spatial chunk
NS = NROW * W              # 392 columns per matmul
P = 128
KC_IN = CIN // P           # 8
MC_W = WIDTH // P          # 2
MC_OUT = COUT // P         # 8
EPS = 1e-5

F32 = mybir.dt.float32
BF16 = mybir.dt.bfloat16
Relu = mybir.ActivationFunctionType.Relu
ADD = mybir.AluOpType.add
MAX = mybir.AluOpType.max

BF16_NP = np.dtype(ml_dtypes.bfloat16)
MM_MODE = "bf16"  # informational; test.py prints this

_NC_CACHE = {}
LAST_RESULT = None  # test.py reads exec_time_ns off this


def _split_multi_waits(nc, maxw=1):
    """walrus codegen rejects instructions carrying more than a couple of
    sem waits ("Too many sync wait commands"); hoist excess waits onto
    same-engine NOPs emitted just before the instruction."""
    for f in nc.m.functions:
        for blk in f.blocks:
            out = []
            changed = False
            for inst in blk.instructions:
                si = inst.sync_info
                if si is not None and len(si.on_wait) > maxw:
                    waits = list(si.on_wait)
                    head, keep = waits[:-maxw], waits[-maxw:]
                    for i in range(0, len(head), maxw):
                        nop = mybir.InstNoOp(
                            name=f"{inst.name}_waitsplit_{i}", ins=[], outs=[]
                        )
                        nop.engine = inst.engine
                        nop.sync_info = mybir.SyncInfo(
                            on_wait=head[i:i + maxw], on_update=[]
                        )
                        out.append(nop)
                    inst.sync_info = mybir.SyncInfo(
                        on_wait=keep, on_update=list(si.on_update)
                    )
                    changed = True
                out.append(inst)
            if changed:
                blk.instructions = out


def _build_nc():
    nc = bass.Bass()
    x_d = nc.dram_tensor("x", [BPC, P, KC_IN, S], BF16, kind="ExternalInput")
    w1_d = nc.dram_tensor("w1", [P, KC_IN, MC_W, P], BF16, kind="ExternalInput")
    w2_d = nc.dram_tensor("w2", [P, 9, MC_W, MC_W, P], BF16, kind="ExternalInput")
    w3_d = nc.dram_tensor("w3", [P, MC_W, MC_OUT, P], BF16, kind="ExternalInput")
    s1_d = nc.dram_tensor("s1", [P, MC_W], F32, kind="ExternalInput")
    s2_d = nc.dram_tensor("s2", [P, MC_W], F32, kind="ExternalInput")
    s3_d = nc.dram_tensor("s3", [P, MC_OUT], F32, kind="ExternalInput")
    id_d = nc.dram_tensor("ident", [P, P], BF16, kind="ExternalInput")
    o_d = nc.dram_tensor("o", [BPC, MC_OUT, P, S], BF16, kind="ExternalOutput")

    with tile.TileContext(nc) as tc:
        with (
            tc.tile_pool(name="consts", bufs=1) as cpool,
            tc.tile_pool(name="xin", bufs=4) as xpool,
            tc.tile_pool(name="a1p", bufs=2) as a1pool,
            tc.tile_pool(name="a2p", bufs=2) as a2pool,
            tc.tile_pool(name="otp", bufs=18) as opool,
            tc.tile_pool(name="ttp", bufs=4) as tpool,
            tc.tile_pool(name="psp", bufs=8, space="PSUM") as pspool,
        ):
            w1_sb = cpool.tile([P, KC_IN, MC_W, P], BF16, tag="w1")
            w2_sb = cpool.tile([P, 9, MC_W, MC_W, P], BF16, tag="w2")
            w3_sb = cpool.tile([P, MC_W, MC_OUT, P], BF16, tag="w3")
            s1_sb = cpool.tile([P, MC_W], F32, tag="s1")
            s2_sb = cpool.tile([P, MC_W], F32, tag="s2")
            s3_sb = cpool.tile([P, MC_OUT], F32, tag="s3")
            # Pre-warm the PE during the DMA lead-in: HAM starts the PE
            # throttled at 1.2 GHz and needs ~3.4us of sustained activity to
            # un-gate; dummy matmuls (no DMA dependency) get that out of the
            # way before the first real matmul's operands land.
            warm_sb = cpool.tile([P, P, 4], BF16, tag="warm")
            nc.vector.memset(warm_sb[:], 0.0)
            ident_sb = cpool.tile([P, P], BF16, tag="ident")
            for _ in range(48):
                wps = pspool.tile([P, 64], F32, tag="ps", name="wps")
                nc.tensor.matmul(wps[:], warm_sb[:, :, 0], warm_sb[:, :64, 0],
                                 start=True, stop=True)
            # a few wide warmups bridge until the first operands land so the
            # first real matmuls run at the un-throttled clock
            for _ in range(2):
                wps = pspool.tile([P, NS], F32, tag="ps", name="wps")
                nc.tensor.matmul(wps[:], warm_sb[:, :, 0],
                                 warm_sb.rearrange("p a b -> p (a b)")[:, :NS],
                                 start=True, stop=True)

            xs = {}      # t -> bf16 [P, KC_IN, S] tile
            a1s = {}     # t -> padded act1 [P, MC_W, PS] bf16
            a2s = {}     # t -> act2 [P, MC_W, S] bf16

            def load(t):
                # one whole-image DMA: 12.5KB contiguous per-partition lines
                # give near-peak HBM packets, and the pipeline gives it a full
                # image of slack so arrival pacing doesn't matter.
                xf = xpool.tile([P, KC_IN, S], BF16, tag="xf")
                nc.sync.dma_start(xf[:], x_d[t])
                xs[t] = xf

            def conv1(t):
                # generator: yields once per PSUM group so the caller can
                # weave conv3 groups between them
                a1 = a1pool.tile([P, MC_W, PS], BF16, tag="a1")
                a14 = a1.rearrange("p m (r c) -> p m r c", c=PW)
                # Zero the pad border on VectorE (cheap strided memsets keep
                # ScalarE free for the PSUM evacuations): top+bottom rows in
                # one op, left+right columns in another.
                for mc in range(MC_W):
                    nc.vector.memset(a14[:, mc, 0:PW:PW - 1, :], 0.0)
                    nc.vector.memset(a14[:, mc, 1:PW - 1, 0:PW:PW - 1], 0.0)
                a1s[t] = a1
                xr = xs[t]
                for sc in range(2):
                    r0 = sc * NROW
                    for mc in range(MC_W):
                        ps = pspool.tile([P, NS], F32, tag="ps")
                        for kc in range(KC_IN):
                            nc.tensor.matmul(
                                ps[:],
                                w1_sb[:, kc, mc],
                                xr[:, kc, sc * NS:(sc + 1) * NS],
                                start=(kc == 0),
                                stop=(kc == KC_IN - 1),
                            )
                        psr = ps.rearrange("p (r c) -> p r c", c=W)
                        nc.scalar.activation(
                            a14[:, mc, 1 + r0:1 + r0 + NROW, 1:1 + W],
                            psr,
                            Relu,
                            bias=s1_sb[:, mc:mc + 1],
                        )
                        yield

            def conv2(t):
                a2 = a2pool.tile([P, MC_W, S], BF16, tag="a2")
                a2s[t] = a2
                a14 = a1s[t].rearrange("p m (r c) -> p m r c", c=PW)
                for sc in range(2):
                    r0 = sc * NROW
                    for mc in range(MC_W):
                        ps = pspool.tile([P, NS], F32, tag="ps")
                        idx = 0
                        for d in range(9):
                            dy, dx = d // 3, d % 3
                            for kc in range(MC_W):
                                nc.tensor.matmul(
                                    ps[:],
                                    w2_sb[:, d, kc, mc],
                                    a14[:, kc, r0 + dy:r0 + dy + NROW, dx:dx + W],
                                    start=(idx == 0),
                                    stop=(idx == 9 * MC_W - 1),
                                )
                                idx += 1
                        nc.scalar.activation(
                            a2[:, mc, sc * NS:(sc + 1) * NS],
                            ps[:],
                            Relu,
                            bias=s2_sb[:, mc:mc + 1],
                        )
                        if sc == 1 and mc == MC_W - 1:
                            del a1s[t]
                        yield

            def conv3(t):
                # sc-outer; one whole-image output push per mc on the Sync
                # queue once its sc=1 half is evacuated
                a2r = a2s[t]
                xf = xs[t]
                osbs = [
                    opool.tile([P, S], BF16, tag="osb", name="osb")
                    for _ in range(MC_OUT)
                ]
                for sc in range(2):
                    for mc in range(MC_OUT):
                        osb = osbs[mc]
                        ps = pspool.tile([P, NS], F32, tag="ps")
                        for kc in range(MC_W):
                            nc.tensor.matmul(
                                ps[:],
                                w3_sb[:, kc, mc],
                                a2r[:, kc, sc * NS:(sc + 1) * NS],
                                start=(kc == 0),
                                stop=(kc == MC_W - 1),
                            )
                        tt = tpool.tile([P, NS], BF16, tag="tt")
                        nc.vector.scalar_tensor_tensor(
                            tt[:],
                            ps[:],
                            s3_sb[:, mc:mc + 1],
                            xf[:, mc, sc * NS:(sc + 1) * NS],
                            ADD,
                            ADD,
                        )
                        nc.scalar.activation(
                            osb[:, sc * NS:(sc + 1) * NS], tt[:], Relu
                        )
                        if sc == 1:
                            nc.sync.dma_start(o_d[t, mc], osb[:])
                        yield
                del a2s[t], xs[t]

            def conv3_last(t):
                # Final image: the residual add folds into the PE (identity x
                # matmul into the same PSUM group) so each chunk drains with
                # one evacuation op, alternating VectorE/ScalarE by mc.
                # sc-outer so its sc0 groups can be woven right behind
                # conv2(t)'s sc0 groups, and each half streams out as soon as
                # it's evacuated (push queues alternate with the evac engine).
                a2r = a2s[t]
                xf = xs[t]
                osbs = [
                    opool.tile([P, S], BF16, tag="osb", name="osb")
                    for _ in range(MC_OUT)
                ]
                for sc in range(2):
                    for mc in range(MC_OUT):
                        osb = osbs[mc]
                        ps = pspool.tile([P, NS], F32, tag="ps")
                        for kc in range(MC_W):
                            nc.tensor.matmul(
                                ps[:],
                                w3_sb[:, kc, mc],
                                a2r[:, kc, sc * NS:(sc + 1) * NS],
                                start=(kc == 0),
                                stop=False,
                            )
                        nc.tensor.matmul(
                            ps[:],
                            ident_sb[:],
                            xf[:, mc, sc * NS:(sc + 1) * NS],
                            start=False,
                            stop=True,
                        )
                        if mc % 2 == 1:
                            nc.vector.tensor_scalar(
                                osb[:, sc * NS:(sc + 1) * NS],
                                ps[:],
                                s3_sb[:, mc:mc + 1],
                                0.0,
                                ADD,
                                MAX,
                            )
                            nc.sync.dma_start(
                                o_d[t, mc, :, sc * NS:(sc + 1) * NS],
                                osb[:, sc * NS:(sc + 1) * NS],
                            )
                        else:
                            nc.scalar.activation(
                                osb[:, sc * NS:(sc + 1) * NS],
                                ps[:],
                                Relu,
                                bias=s3_sb[:, mc:mc + 1],
                            )
                            nc.scalar.dma_start(
                                o_d[t, mc, :, sc * NS:(sc + 1) * NS],
                                osb[:, sc * NS:(sc + 1) * NS],
                            )
                        yield
                del a2s[t], xs[t]

            # Startup DMA priority order: strictly what the PE needs next.
            # w1/x0 interleaved per-K-chunk (conv1(0) paces off arrivals),
            # then w2 in 6 chunks ordered exactly as conv2(0) consumes them
            # (d-third outer, kc inner), then x(1), w3.
            xf0 = xpool.tile([P, KC_IN, S], BF16, tag="xf", name="xf")
            nc.sync.dma_start(w1_sb[:, 0:1], w1_d[:, 0:1])
            nc.sync.dma_start(xf0[:, 0:1], x_d[0, :, 0:1])
            nc.sync.dma_start(s1_sb[:], s1_d[:])
            nc.sync.dma_start(w1_sb[:, 1:8], w1_d[:, 1:8])
            nc.sync.dma_start(xf0[:, 1:2], x_d[0, :, 1:2])
            nc.sync.dma_start(xf0[:, 2:4], x_d[0, :, 2:4])
            nc.sync.dma_start(xf0[:, 4:6], x_d[0, :, 4:6])
            nc.sync.dma_start(xf0[:, 6:8], x_d[0, :, 6:8])
            xs[0] = xf0
            nc.sync.dma_start(s2_sb[:], s2_d[:])
            for i3 in range(3):
                nc.sync.dma_start(
                    w2_sb[:, 3 * i3:3 * i3 + 3],
                    w2_d[:, 3 * i3:3 * i3 + 3],
                )
            load(1)
            nc.sync.dma_start(w3_sb[:], w3_d[:])
            nc.sync.dma_start(s3_sb[:], s3_d[:])
            nc.sync.dma_start(ident_sb[:], id_d[:])

            # conv2(t-2) groups are issued before conv1(t-1) groups: during
            # the lead-in conv2(0)'s weights arrive well before conv1(1)'s
            # image, so this order keeps the PE fed (and HAM un-throttled)
            # through the fill. conv3(t-3)'s groups are woven 2-per-group
            # between them: conv3's matmul bursts are short relative to their
            # VectorE evacuations, so run back-to-back they stall the PE on
            # PSUM recycling; interleaved they never wait.
            def weave(big, small, ratio=2):
                for g in big:
                    for _ in g:
                        for _ in range(ratio):
                            if small is not None and next(small, "end") == "end":
                                small = None
                if small is not None:
                    for _ in small:
                        pass

            def chain(*gens):
                for g in gens:
                    yield from g

            for t in range(BPC + 2):
                if 1 < t < BPC:
                    load(t)
                big = []
                if 0 <= t - 2 < BPC:
                    big.append(conv2(t - 2))
                if 0 <= t - 1 < BPC:
                    big.append(conv1(t - 1))
                if t == BPC + 1:
                    # final weave: conv2(last) has only 4 groups left to hide
                    # conv3(last-1)'s 16 and conv3_last(last)'s 16 behind, so
                    # pull 8 per group; the ordering works out so every
                    # conv3_last group's a2 chunks are complete when issued.
                    small = chain(conv3(t - 3), conv3_last(t - 2))
                    weave(big, small, ratio=8)
                else:
                    small = conv3(t - 3) if 0 <= t - 3 < BPC else None
                    weave(big, small)

    return nc


def _prep_inputs(x, w1, w2, w3, g1, b1, m1, v1, g2, b2, m2, v2, g3, b3, m3, v3):
    """Fold BN into weights/shifts and pack everything into per-core maps."""

    def fold(wv, g, bb, m, v):
        inv = (g / np.sqrt(v + EPS)).astype(np.float32)
        shift = (bb - m * inv).astype(np.float32)
        return wv * inv[:, None, None, None], shift

    w1f, sh1 = fold(np.asarray(w1, np.float32), g1, b1, m1, v1)
    w2f, sh2 = fold(np.asarray(w2, np.float32), g2, b2, m2, v2)
    w3f, sh3 = fold(np.asarray(w3, np.float32), g3, b3, m3, v3)

    # lhsT layouts: partition = K-within-chunk, free = [kc?, d?, mc, m]
    w1h = np.ascontiguousarray(
        w1f[:, :, 0, 0].T.reshape(KC_IN, P, MC_W, P).transpose(1, 0, 2, 3)
    ).astype(BF16_NP)
    w2h = np.ascontiguousarray(
        w2f.transpose(2, 3, 1, 0)           # [ky, kx, in, out]
        .reshape(9, MC_W, P, MC_W, P)
        .transpose(2, 0, 1, 3, 4)
    ).astype(BF16_NP)
    w3h = np.ascontiguousarray(
        w3f[:, :, 0, 0].T.reshape(MC_W, P, MC_OUT, P).transpose(1, 0, 2, 3)
    ).astype(BF16_NP)
    s1h = np.ascontiguousarray(sh1.reshape(MC_W, P).T)
    s2h = np.ascontiguousarray(sh2.reshape(MC_W, P).T)
    s3h = np.ascontiguousarray(sh3.reshape(MC_OUT, P).T)

    xnp = np.asarray(x, np.float32).astype(BF16_NP)
    in_maps = []
    for c in range(NCORES):
        xc = np.ascontiguousarray(
            xnp[c * BPC:(c + 1) * BPC]
            .reshape(BPC, KC_IN, P, S)
            .transpose(0, 2, 1, 3)
        )
        in_maps.append({
            "x": xc, "w1": w1h, "w2": w2h, "w3": w3h,
            "s1": s1h, "s2": s2h, "s3": s3h,
            "ident": np.eye(P, dtype=BF16_NP),
        })
    return in_maps


def _ensure_ntff_hook():
    """If tracing is requested but this image's antenv lacks axon_hooks,
    register an in-process shim (or disable tracing) so run_bass_kernel_spmd
    doesn't crash on the import."""
    if os.environ.get("BASS_TRACE") != "1":
        return
    try:
        import antenv.axon_hooks  # noqa: F401
        return
    except ImportError:
        pass
    try:
        import sys
        import types
        import antenv
        from trn_agent_boot.trn_boot import _ntff_profile_via_ctypes

        hook = _ntff_profile_via_ctypes("/opt/axon/libaxon_pjrt.so")
        mod = types.ModuleType("antenv.axon_hooks")
        state = {"hook": hook}
        mod.set_axon_ntff_profile_hook = lambda h: state.__setitem__("hook", h)
        mod.get_axon_ntff_profile_hook = lambda: state["hook"]
        antenv.axon_hooks = mod
        sys.modules["antenv.axon_hooks"] = mod
    except Exception:
        os.environ["BASS_NEVER_TRACE"] = "1"


def kernel(**inputs):
    global LAST_RESULT
    _ensure_ntff_hook()
    if "nc" not in _NC_CACHE:
        nc = _build_nc()
        _split_multi_waits(nc)  # HW-only legalization; CoreSim can't run it
        _NC_CACHE["nc"] = nc
    nc = _NC_CACHE["nc"]
    in_maps = _prep_inputs(**inputs)
    res = run_bass_kernel_spmd(nc, in_maps, list(range(NCORES)))
    LAST_RESULT = res
    out = np.concatenate([r["o"] for r in res.results], axis=0)
    return np.ascontiguousarray(
        out.reshape(B, COUT, H, W).astype(np.float32)
    )


# revision 33
# speedup vs baseline: 1.0024x; 1.0024x over previous
"""ResNet bottleneck block (1x1 -> 3x3 -> 1x1 convs, folded BN, residual ReLU)
on 8 Trainium2 NeuronCores, data-parallel over the batch dim.

Layout strategy (per core, 8 images):
  - x arranged [img, p, kc, hw] so each image DMA is one contiguous
    [128, 8*784] transfer; channel c = kc*128 + p.
  - All matmul operands are bf16 (weights folded+cast host-side, x cast
    host-side): bf16 LDWEIGHTS uses the fast-weight-load path (~53ns) and
    hides behind the 163ns matmul stream, where f32r's 2-pass ~225ns weight
    load serialized ahead of every matmul and set the PE cadence.
  - 1x1 convs are matmuls over the flattened spatial dim (N split 2x392).
  - 3x3 conv = 9 shifted matmuls accumulating in PSUM, reading a
    zero-padded 30x30 SBUF image so every matmul is a uniform [128,14,28].
  - BN scale folded into the weights host-side; BN shift + ReLU fused into
    the PSUM->SBUF evacuation on ScalarE. conv3's residual-add runs as a
    scalar_tensor_tensor on VectorE, final ReLU on ScalarE.
  - Output stored bf16 and upcast to f32 host-side (halves the store DMA).
  - Software pipeline over images: DMA(t) / conv1(t-1) / conv2(t-2) /
    conv3+store(t-3) so the PE stream never waits on a same-image epilogue.
"""

import math
import os

import numpy as np
import ml_dtypes

import concourse.bass as bass
import concourse.mybir as mybir
import concourse.tile as tile
from concourse.bass_utils import run_bass_kernel_spmd

# Problem constants (hardcoded per the grading contract).
B, CIN, H, W = 64, 1024, 28, 28
WIDTH, COUT = 256, 1024
NCORES = 8
BPC = B // NCORES          # images per core
S = H * W                  # 784
PW = W + 2                 # 30 (padded row width)
PS = PW * PW               # 900
NROW = H // 2              # 14 rows per spatial chunk
NS = NROW * W              # 392 columns per matmul
P = 128
KC_IN = CIN // P           # 8
MC_W = WIDTH // P          # 2
MC_OUT = COUT // P         # 8
EPS = 1e-5

F32 = mybir.dt.float32
BF16 = mybir.dt.bfloat16
Relu = mybir.ActivationFunctionType.Relu
ADD = mybir.AluOpType.add
MAX = mybir.AluOpType.max

BF16_NP = np.dtype(ml_dtypes.bfloat16)
MM_MODE = "bf16"  # informational; test.py prints this

_NC_CACHE = {}
LAST_RESULT = None  # test.py reads exec_time_ns off this


def _split_multi_waits(nc, maxw=1):
    """walrus codegen rejects instructions carrying more than a couple of
    sem waits ("Too many sync wait commands"); hoist excess waits onto
    same-engine NOPs emitted just before the instruction."""
    for f in nc.m.functions:
        for blk in f.blocks:
            out = []
            changed = False
            for inst in blk.instructions:
                si = inst.sync_info
                if si is not None and len(si.on_wait) > maxw:
                    waits = list(si.on_wait)
                    head, keep = waits[:-maxw], waits[-maxw:]
                    for i in range(0, len(head), maxw):
                        nop = mybir.InstNoOp(
                            name=f"{inst.name}_waitsplit_{i}", ins=[], outs=[]
                        )
                        nop.engine = inst.engine
                        nop.sync_info = mybir.SyncInfo(
                            on_wait=head[i:i + maxw], on_update=[]
                        )
                        out.append(nop)
                    inst.sync_info = mybir.SyncInfo(
                        on_wait=keep, on_update=list(si.on_update)
                    )
                    changed = True
                out.append(inst)
            if changed:
                blk.instructions = out


def _build_nc():
    nc = bass.Bass()
    x_d = nc.dram_tensor("x", [BPC, P, KC_IN, S], BF16, kind="ExternalInput")
    w1_d = nc.dram_tensor("w1", [P, KC_IN, MC_W, P], BF16, kind="ExternalInput")
    w2_d = nc.dram_tensor("w2", [P, 9, MC_W, MC_W, P], BF16, kind="ExternalInput")
    w3_d = nc.dram_tensor("w3", [P, MC_W, MC_OUT, P], BF16, kind="ExternalInput")
    s1_d = nc.dram_tensor("s1", [P, MC_W], F32, kind="ExternalInput")
    s2_d = nc.dram_tensor("s2", [P, MC_W], F32, kind="ExternalInput")
    s3_d = nc.dram_tensor("s3", [P, MC_OUT], F32, kind="ExternalInput")
    id_d = nc.dram_tensor("ident", [P, P], BF16, kind="ExternalInput")
    o_d = nc.dram_tensor("o", [BPC, MC_OUT, P, S], BF16, kind="ExternalOutput")

    with tile.TileContext(nc) as tc:
        with (
            tc.tile_pool(name="consts", bufs=1) as cpool,
            tc.tile_pool(name="xin", bufs=4) as xpool,
            tc.tile_pool(name="a1p", bufs=2) as a1pool,
            tc.tile_pool(name="a2p", bufs=2) as a2pool,
            tc.tile_pool(name="otp", bufs=18) as opool,
            tc.tile_pool(name="ttp", bufs=4) as tpool,
            tc.tile_pool(name="psp", bufs=8, space="PSUM") as pspool,
        ):
            w1_sb = cpool.tile([P, KC_IN, MC_W, P], BF16, tag="w1")
            w2_sb = cpool.tile([P, 9, MC_W, MC_W, P], BF16, tag="w2")
            w3_sb = cpool.tile([P, MC_W, MC_OUT, P], BF16, tag="w3")
            s1_sb = cpool.tile([P, MC_W], F32, tag="s1")
            s2_sb = cpool.tile([P, MC_W], F32, tag="s2")
            s3_sb = cpool.tile([P, MC_OUT], F32, tag="s3")
            # Pre-warm the PE during the DMA lead-in: HAM starts the PE
            # throttled at 1.2 GHz and needs ~3.4us of sustained activity to
            # un-gate; dummy matmuls (no DMA dependency) get that out of the
            # way before the first real matmul's operands land.
            warm_sb = cpool.tile([P, P, 4], BF16, tag="warm")
            nc.vector.memset(warm_sb[:], 0.0)
            ident_sb = cpool.tile([P, P], BF16, tag="ident")
            for _ in range(56):
                wps = pspool.tile([P, 64], F32, tag="ps", name="wps")
                nc.tensor.matmul(wps[:], warm_sb[:, :, 0], warm_sb[:, :64, 0],
                                 start=True, stop=True)
            # a few wide warmups bridge until the first operands land so the
            # first real matmuls run at the un-throttled clock
            for _ in range(2):
                wps = pspool.tile([P, NS], F32, tag="ps", name="wps")
                nc.tensor.matmul(wps[:], warm_sb[:, :, 0],
                                 warm_sb.rearrange("p a b -> p (a b)")[:, :NS],
                                 start=True, stop=True)

            xs = {}      # t -> bf16 [P, KC_IN, S] tile
            a1s = {}     # t -> padded act1 [P, MC_W, PS] bf16
            a2s = {}     # t -> act2 [P, MC_W, S] bf16

            def load(t):
                # one whole-image DMA: 12.5KB contiguous per-partition lines
                # give near-peak HBM packets, and the pipeline gives it a full
                # image of slack so arrival pacing doesn't matter.
                xf = xpool.tile([P, KC_IN, S], BF16, tag="xf")
                nc.sync.dma_start(xf[:], x_d[t])
                xs[t] = xf

            def conv1(t):
                # generator: yields once per PSUM group so the caller can
                # weave conv3 groups between them
                a1 = a1pool.tile([P, MC_W, PS], BF16, tag="a1")
                a14 = a1.rearrange("p m (r c) -> p m r c", c=PW)
                # Zero the pad border on VectorE (cheap strided memsets keep
                # ScalarE free for the PSUM evacuations): top+bottom rows in
                # one op, left+right columns in another.
                for mc in range(MC_W):
                    nc.vector.memset(a14[:, mc, 0:PW:PW - 1, :], 0.0)
                    nc.vector.memset(a14[:, mc, 1:PW - 1, 0:PW:PW - 1], 0.0)
                a1s[t] = a1
                xr = xs[t]
                for sc in range(2):
                    r0 = sc * NROW
                    for mc in range(MC_W):
                        ps = pspool.tile([P, NS], F32, tag="ps")
                        for kc in range(KC_IN):
                            nc.tensor.matmul(
                                ps[:],
                                w1_sb[:, kc, mc],
                                xr[:, kc, sc * NS:(sc + 1) * NS],
                                start=(kc == 0),
                                stop=(kc == KC_IN - 1),
                            )
                        psr = ps.rearrange("p (r c) -> p r c", c=W)
                        nc.scalar.activation(
                            a14[:, mc, 1 + r0:1 + r0 + NROW, 1:1 + W],
                            psr,
                            Relu,
                            bias=s1_sb[:, mc:mc + 1],
                        )
                        yield

            def conv2(t):
                a2 = a2pool.tile([P, MC_W, S], BF16, tag="a2")
                a2s[t] = a2
                a14 = a1s[t].rearrange("p m (r c) -> p m r c", c=PW)
                for sc in range(2):
                    r0 = sc * NROW
                    for mc in range(MC_W):
                        ps = pspool.tile([P, NS], F32, tag="ps")
                        idx = 0
                        for d in range(9):
                            dy, dx = d // 3, d % 3
                            for kc in range(MC_W):
                                nc.tensor.matmul(
                                    ps[:],
                                    w2_sb[:, d, kc, mc],
                                    a14[:, kc, r0 + dy:r0 + dy + NROW, dx:dx + W],
                                    start=(idx == 0),
                                    stop=(idx == 9 * MC_W - 1),
                                )
                                idx += 1
                        nc.scalar.activation(
                            a2[:, mc, sc * NS:(sc + 1) * NS],
                            ps[:],
                            Relu,
                            bias=s2_sb[:, mc:mc + 1],
                        )
                        if sc == 1 and mc == MC_W - 1:
                            del a1s[t]
                        yield

            def conv3(t):
                # sc-outer; one whole-image output push per mc on the Sync
                # queue once its sc=1 half is evacuated
                a2r = a2s[t]
                xf = xs[t]
                osbs = [
                    opool.tile([P, S], BF16, tag="osb", name="osb")
                    for _ in range(MC_OUT)
                ]
                for sc in range(2):
                    for mc in range(MC_OUT):
                        osb = osbs[mc]
                        ps = pspool.tile([P, NS], F32, tag="ps")
                        for kc in range(MC_W):
                            nc.tensor.matmul(
                                ps[:],
                                w3_sb[:, kc, mc],
                                a2r[:, kc, sc * NS:(sc + 1) * NS],
                                start=(kc == 0),
                                stop=(kc == MC_W - 1),
                            )
                        tt = tpool.tile([P, NS], BF16, tag="tt")
                        nc.vector.scalar_tensor_tensor(
                            tt[:],
                            ps[:],
                            s3_sb[:, mc:mc + 1],
                            xf[:, mc, sc * NS:(sc + 1) * NS],
                            ADD,
                            ADD,
                        )
                        nc.scalar.activation(
                            osb[:, sc * NS:(sc + 1) * NS], tt[:], Relu
                        )
                        if sc == 1:
                            nc.sync.dma_start(o_d[t, mc], osb[:])
                        yield
                del a2s[t], xs[t]

            def conv3_last(t):
                # Final image: the residual add folds into the PE (identity x
                # matmul into the same PSUM group) so each chunk drains with
                # one evacuation op, alternating VectorE/ScalarE by mc.
                # sc-outer so its sc0 groups can be woven right behind
                # conv2(t)'s sc0 groups, and each half streams out as soon as
                # it's evacuated (push queues alternate with the evac engine).
                a2r = a2s[t]
                xf = xs[t]
                osbs = [
                    opool.tile([P, S], BF16, tag="osb", name="osb")
                    for _ in range(MC_OUT)
                ]
                for sc in range(2):
                    for mc in range(MC_OUT):
                        osb = osbs[mc]
                        ps = pspool.tile([P, NS], F32, tag="ps")
                        for kc in range(MC_W):
                            nc.tensor.matmul(
                                ps[:],
                                w3_sb[:, kc, mc],
                                a2r[:, kc, sc * NS:(sc + 1) * NS],
                                start=(kc == 0),
                                stop=False,
                            )
                        nc.tensor.matmul(
                            ps[:],
                            ident_sb[:],
                            xf[:, mc, sc * NS:(sc + 1) * NS],
                            start=False,
                            stop=True,
                        )
                        if mc % 2 == 1:
                            nc.vector.tensor_scalar(
                                osb[:, sc * NS:(sc + 1) * NS],
                                ps[:],
                                s3_sb[:, mc:mc + 1],
                                0.0,
                                ADD,
                                MAX,
                            )
                            nc.sync.dma_start(
                                o_d[t, mc, :, sc * NS:(sc + 1) * NS],
                                osb[:, sc * NS:(sc + 1) * NS],
                            )
                        else:
                            nc.scalar.activation(
                                osb[:, sc * NS:(sc + 1) * NS],
                                ps[:],
                                Relu,
                                bias=s3_sb[:, mc:mc + 1],
                            )
                            nc.scalar.dma_start(
                                o_d[t, mc, :, sc * NS:(sc + 1) * NS],
                                osb[:, sc * NS:(sc + 1) * NS],
                            )
                        yield
                del a2s[t], xs[t]

            # Startup DMA priority order: strictly what the PE needs next.
            # w1/x0 interleaved per-K-chunk (conv1(0) paces off arrivals),
            # then w2 in 6 chunks ordered exactly as conv2(0) consumes them
            # (d-third outer, kc inner), then x(1), w3.
            xf0 = xpool.tile([P, KC_IN, S], BF16, tag="xf", name="xf")
            nc.sync.dma_start(w1_sb[:, 0:1], w1_d[:, 0:1])
            nc.sync.dma_start(xf0[:, 0:1], x_d[0, :, 0:1])
            nc.sync.dma_start(s1_sb[:], s1_d[:])
            nc.sync.dma_start(w1_sb[:, 1:8], w1_d[:, 1:8])
            nc.sync.dma_start(xf0[:, 1:2], x_d[0, :, 1:2])
            nc.sync.dma_start(xf0[:, 2:4], x_d[0, :, 2:4])
            nc.sync.dma_start(xf0[:, 4:6], x_d[0, :, 4:6])
            nc.sync.dma_start(xf0[:, 6:8], x_d[0, :, 6:8])
            xs[0] = xf0
            nc.sync.dma_start(s2_sb[:], s2_d[:])
            for i3 in range(3):
                nc.sync.dma_start(
                    w2_sb[:, 3 * i3:3 * i3 + 3],
                    w2_d[:, 3 * i3:3 * i3 + 3],
                )
            load(1)
            nc.sync.dma_start(w3_sb[:], w3_d[:])
            nc.sync.dma_start(s3_sb[:], s3_d[:])
            nc.sync.dma_start(ident_sb[:], id_d[:])

            # conv2(t-2) groups are issued before conv1(t-1) groups: during
            # the lead-in conv2(0)'s weights arrive well before conv1(1)'s
            # image, so this order keeps the PE fed (and HAM un-throttled)
            # through the fill. conv3(t-3)'s groups are woven 2-per-group
            # between them: conv3's matmul bursts are short relative to their
            # VectorE evacuations, so run back-to-back they stall the PE on
            # PSUM recycling; interleaved they never wait.
            def weave(big, small, ratio=2):
                for g in big:
                    for _ in g:
                        for _ in range(ratio):
                            if small is not None and next(small, "end") == "end":
                                small = None
                if small is not None:
                    for _ in small:
                        pass

            def chain(*gens):
                for g in gens:
                    yield from g

            for t in range(BPC + 2):
                if 1 < t < BPC:
                    load(t)
                big = []
                if 0 <= t - 2 < BPC:
                    big.append(conv2(t - 2))
                if 0 <= t - 1 < BPC:
                    big.append(conv1(t - 1))
                if t == BPC + 1:
                    # final weave: conv2(last) has only 4 groups left to hide
                    # conv3(last-1)'s 16 and conv3_last(last)'s 16 behind, so
                    # pull 8 per group; the ordering works out so every
                    # conv3_last group's a2 chunks are complete when issued.
                    small = chain(conv3(t - 3), conv3_last(t - 2))
                    weave(big, small, ratio=8)
                else:
                    small = conv3(t - 3) if 0 <= t - 3 < BPC else None
                    weave(big, small)

    return nc


def _prep_inputs(x, w1, w2, w3, g1, b1, m1, v1, g2, b2, m2, v2, g3, b3, m3, v3):
    """Fold BN into weights/shifts and pack everything into per-core maps."""

    def fold(wv, g, bb, m, v):
        inv = (g / np.sqrt(v + EPS)).astype(np.float32)
        shift = (bb - m * inv).astype(np.float32)
        return wv * inv[:, None, None, None], shift

    w1f, sh1 = fold(np.asarray(w1, np.float32), g1, b1, m1, v1)
    w2f, sh2 = fold(np.asarray(w2, np.float32), g2, b2, m2, v2)
    w3f, sh3 = fold(np.asarray(w3, np.float32), g3, b3, m3, v3)

    # lhsT layouts: partition = K-within-chunk, free = [kc?, d?, mc, m]
    w1h = np.ascontiguousarray(
        w1f[:, :, 0, 0].T.reshape(KC_IN, P, MC_W, P).transpose(1, 0, 2, 3)
    ).astype(BF16_NP)
    w2h = np.ascontiguousarray(
        w2f.transpose(2, 3, 1, 0)           # [ky, kx, in, out]
        .reshape(9, MC_W, P, MC_W, P)
        .transpose(2, 0, 1, 3, 4)
    ).astype(BF16_NP)
    w3h = np.ascontiguousarray(
        w3f[:, :, 0, 0].T.reshape(MC_W, P, MC_OUT, P).transpose(1, 0, 2, 3)
    ).astype(BF16_NP)
    s1h = np.ascontiguousarray(sh1.reshape(MC_W, P).T)
    s2h = np.ascontiguousarray(sh2.reshape(MC_W, P).T)
    s3h = np.ascontiguousarray(sh3.reshape(MC_OUT, P).T)

    xnp = np.asarray(x, np.float32).astype(BF16_NP)
    in_maps = []
    for c in range(NCORES):
        xc = np.ascontiguousarray(
            xnp[c * BPC:(c + 1) * BPC]
            .reshape(BPC, KC_IN, P, S)
            .transpose(0, 2, 1, 3)
        )
        in_maps.append({
            "x": xc, "w1": w1h, "w2": w2h, "w3": w3h,
            "s1": s1h, "s2": s2h, "s3": s3h,
            "ident": np.eye(P, dtype=BF16_NP),
        })
    return in_maps


def _ensure_ntff_hook():
    """If tracing is requested but this image's antenv lacks axon_hooks,
    register an in-process shim (or disable tracing) so run_bass_kernel_spmd
    doesn't crash on the import."""
    if os.environ.get("BASS_TRACE") != "1":
        return
    try:
        import antenv.axon_hooks  # noqa: F401
        return
    except ImportError:
        pass
    try:
        import sys
        import types
        import antenv
        from trn_agent_boot.trn_boot import _ntff_profile_via_ctypes

        hook = _ntff_profile_via_ctypes("/opt/axon/libaxon_pjrt.so")
        mod = types.ModuleType("antenv.axon_hooks")
        state = {"hook": hook}
        mod.set_axon_ntff_profile_hook = lambda h: state.__setitem__("hook", h)
        mod.get_axon_ntff_profile_hook = lambda: state["hook"]
        antenv.axon_hooks = mod
        sys.modules["antenv.axon_hooks"] = mod
    except Exception:
        os.environ["BASS_NEVER_TRACE"] = "1"


def kernel(**inputs):
    global LAST_RESULT
    _ensure_ntff_hook()
    if "nc" not in _NC_CACHE:
        nc = _build_nc()
        _split_multi_waits(nc)  # HW-only legalization; CoreSim can't run it
        _NC_CACHE["nc"] = nc
    nc = _NC_CACHE["nc"]
    in_maps = _prep_inputs(**inputs)
    res = run_bass_kernel_spmd(nc, in_maps, list(range(NCORES)))
    LAST_RESULT = res
    out = np.concatenate([r["o"] for r in res.results], axis=0)
    return np.ascontiguousarray(
        out.reshape(B, COUT, H, W).astype(np.float32)
    )
